# revision 3
# baseline (speedup 1.0000x reference)
"""GRU encoder step (embedding lookup + GRUCell, batch=1) on 8 TRN2 cores.

Sharding: core k computes hidden dims [32k, 32k+32); the host concatenates.

Key mechanisms (chosen from trace analysis):
- The looked-up embedding row is fetched with a register-dynamic HWDGE DMA:
  scalar reg_loads the element offset (idx*HID, host-marshaled i32) straight
  from DRAM and issues a ONE-DESCRIPTOR DMA of table[idx] into a single SBUF
  partition. One descriptor = one DMA engine = no 16-way completion
  rendezvous (a straggling engine was costing 1-2us on sprayed DMAs).
- The idle PE broadcasts the row across partitions: ones[1,64].T @ x[1,257]
  -> psum[64, 257] (col 256 is a 1.0 used to fold b_ih_n into the gxi
  contraction).
- ALL weights/biases/h ride in ONE [128, 773] DMA (first in flight, so its
  16-way completion is ahead of everything else):
    rows 0:32  : [w_hh_n | b_hh_n 0 | h 1 1 | w_ih_z | hs]
    rows 32:64 : [w_ih_r | w_ih_n | b_ih_n | 0...]
    rows 64:96 : [w_hh_r | b_ih_r b_hh_r | h 1 1 | 0...]
    rows 96:128: [w_hh_z | b_ih_z b_hh_z | h 1 1 | 0...]
  One 128-partition STT then yields gh[0:32]=ghn_full (tanh src),
  gh[32:64]=garbage (unused), gh[64:96]=r bias, gh[96:128]=z bias.
- x-side contractions read the PSUM broadcast: gxr/gxi on partitions 32:64,
  gxz on 0:32 (so z_t lands on 0:32 where the final blend runs).
- n = tanh(ghn*r + gxi) via ACT scale/bias APs (+32 partition-shifted).
- Consumer semaphore waits are fused onto the consuming instructions
  (standalone EVENT_SEMAPHORE waits cost ~80ns each).
- No nc.Block(): no entry branches, no bass exit barrier; the walrus
  epilogue's own barrier gates the teardown. Output DMA has no completion
  wait (lands during teardown).
"""

import os
import sys

import numpy as np

for _p in ("/opt/trn_rl_repo",):
    if _p not in sys.path and os.path.isdir(_p):
        sys.path.insert(0, _p)

import concourse.bass as bass
from concourse.ap import AP
from concourse import mybir

VOCAB = 100000
HID = 256
NCORES = 8
G = HID // NCORES  # 32
WD = 3 * HID + 5  # 773

_cached = None


def build_program():
    nc = bass.Bass(
        "TRN2",
        target_bir_lowering=False,
        debug=False,
        enable_asserts=False,
        num_devices=NCORES,
    )
    f32 = mybir.dt.float32
    i32 = mybir.dt.int32

    table_t = nc.dram_tensor("table", [VOCAB, HID], f32, kind="ExternalInput")
    off_t = nc.dram_tensor("off", [1, 1], i32, kind="ExternalInput")
    wd_d = nc.dram_tensor("wd", [128, WD], f32, kind="ExternalInput").ap()
    # [G, 128] so each partition's DMA write is 512B (avoids sub-512B RMW);
    # host reads column 0.
    out_d = nc.dram_tensor("out", [G, 128], f32, kind="ExternalOutput").ap()

    wd_sb = nc.alloc_sbuf_tensor("wd_sb", [128, WD], f32).ap()
    bxa1 = nc.alloc_sbuf_tensor("bxa1", [1, HID + 1], f32).ap()
    ones = nc.alloc_sbuf_tensor("ones", [1, 2 * G], f32).ap()
    px = nc.alloc_psum_tensor("px", [2 * G, HID + 1], f32).ap()
    s1 = nc.alloc_sbuf_tensor("s1", [2 * G, HID], f32).ap()
    s1b = nc.alloc_sbuf_tensor("s1b", [2 * G, HID + 1], f32).ap()
    s2 = nc.alloc_sbuf_tensor("s2", [128, HID + 2], f32).ap()
    s3 = nc.alloc_sbuf_tensor("s3", [G, HID], f32).ap()
    gh = nc.alloc_sbuf_tensor("gh", [128, 1], f32).ap()
    gxr = nc.alloc_sbuf_tensor("gxr", [2 * G, 1], f32).ap()
    gxi = nc.alloc_sbuf_tensor("gxi", [2 * G, 1], f32).ap()
    gxz = nc.alloc_sbuf_tensor("gxz", [G, 1], f32).ap()
    r_t = nc.alloc_sbuf_tensor("r_t", [2 * G, 1], f32).ap()
    z_t = nc.alloc_sbuf_tensor("z_t", [G, 1], f32).ap()
    n_t = nc.alloc_sbuf_tensor("n_t", [G, 1], f32).ap()
    d_t = nc.alloc_sbuf_tensor("d_t", [G, 1], f32).ap()
    out_sb = nc.alloc_sbuf_tensor("out_sb", [G, 128], f32).ap()
    warm = nc.alloc_sbuf_tensor("warm", [G, 1], f32).ap()

    hs_view = wd_sb[0:G, WD - 1 : WD]

    off_reg = nc.scalar.alloc_register("offr")

    with (
        nc.semaphore() as s_gx,
        nc.semaphore() as s_wd,
        nc.semaphore() as s_m,
        nc.semaphore() as s_px,
        nc.semaphore() as s_v,
        nc.semaphore() as s_ve,
        nc.semaphore() as s_s,
        nc.semaphore() as s_out,
    ):
        sync = nc.sync
        gpsimd = nc.gpsimd
        scalar = nc.scalar
        vector = nc.vector
        tensor = nc.tensor

        # --- sync: the single merged weight DMA, first in flight ---
        sync.dma_start(wd_sb[:], wd_d[:]).then_inc(s_wd, 16)

        # --- gpsimd: ones for the PE broadcast + the 1.0 in bxa1 col 256 ---
        gpsimd.memset(ones[:], 1.0)
        gpsimd.memset(bxa1[0:1, HID : HID + 1], 1.0).then_inc(s_m, 1)

        # --- scalar: 1-descriptor gather on the uncontended Act queue ---
        scalar.reg_load(off_reg, off_t[0:1, 0:1])
        v = scalar.snap(off_reg, donate=True, min_val=0, max_val=(VOCAB - 1) * HID)
        dyn = AP(tensor=table_t.ap().tensor, offset=v, ap=[[0, 1], [1, HID]])
        scalar.dma_start(bxa1[0:1, 0:HID], dyn).then_inc(s_gx, 16)
        # Warm the ACT function table while DMAs are in flight.
        const0 = nc.const_aps.aps[(mybir.dt.float32, 0.0)]
        scalar.activation(
            warm[:], const0[0:G, :1], mybir.ActivationFunctionType.Sigmoid
        )

        # --- tensor: broadcast x across 64 partitions via ones-matmul ---
        tensor.wait_ge(s_m, 1)  # ones ready (gather wait fused on matmul)
        # wait >=16: a DMA's completion is 16 per-engine sem packets and only
        # the data-carrying engine's packet is write-after-write ordered with
        # the data -- waiting >=1 races the 15 empty packets (observed
        # intermittent corruption).
        tensor.matmul(
            px[0 : 2 * G, 0 : HID + 1],
            ones[0:1, 0 : 2 * G],
            bxa1[0:1, 0 : HID + 1],
        ).wait_op(s_gx, 16, "sem-ge").then_inc(s_px, 1)

        # --- vector: h-side contraction for all gates in one 128-row STT ---
        vector.scalar_tensor_tensor(
            out=s2[:],
            in0=wd_sb[:, 0 : HID + 2],
            scalar=1.0,
            in1=wd_sb[:, HID + 2 : 2 * HID + 4],
            op0=mybir.AluOpType.mult,
            op1=mybir.AluOpType.mult,
            accum_out=gh[:],
        ).wait_op(s_wd, 16, "sem-ge").then_inc(s_v, 1)
        # x-side contraction, r rows (partitions 32:64, reads PSUM broadcast)
        vector.scalar_tensor_tensor(
            out=s1[G : 2 * G, 0:HID],
            in0=wd_sb[G : 2 * G, 0:HID],
            scalar=1.0,
            in1=px[G : 2 * G, 0:HID],
            op0=mybir.AluOpType.mult,
            op1=mybir.AluOpType.mult,
            accum_out=gxr[G : 2 * G, :],
        ).wait_op(s_px, 1, "sem-ge").then_inc(s_v, 1)

        # r = sigmoid(gxr + gh_r): src/out partitions 32:64, bias at 64:96
        scalar.activation(
            r_t[G : 2 * G, :],
            gxr[G : 2 * G, :1],
            mybir.ActivationFunctionType.Sigmoid,
            bias=gh[2 * G : 3 * G, :1],
        ).wait_op(s_v, 2, "sem-ge").then_inc(s_s, 1)

        # x-side contraction, i_n rows + b_ih_n via psum ones column
        vector.scalar_tensor_tensor(
            out=s1b[G : 2 * G, :],
            in0=wd_sb[G : 2 * G, HID : 2 * HID + 1],
            scalar=1.0,
            in1=px[G : 2 * G, 0 : HID + 1],
            op0=mybir.AluOpType.mult,
            op1=mybir.AluOpType.mult,
            accum_out=gxi[G : 2 * G, :],
        ).then_inc(s_v, 1)
        # x-side contraction, z rows (partitions 0:32)
        vector.scalar_tensor_tensor(
            out=s3[:],
            in0=wd_sb[0:G, 2 * HID + 4 : 3 * HID + 4],
            scalar=1.0,
            in1=px[0:G, 0:HID],
            op0=mybir.AluOpType.mult,
            op1=mybir.AluOpType.mult,
            accum_out=gxz[:],
        ).then_inc(s_v, 1)

        # n = tanh(ghn*r + gxi): src gh[0:32], scale r_t(+32), bias gxi(+32)
        scalar.wait_ge(s_s, 1)  # sem edge for r_t (same-engine RAW)
        scalar.activation(
            n_t[:],
            gh[0:G, :1],
            mybir.ActivationFunctionType.Tanh,
            bias=gxi[G : 2 * G, :1],
            scale=r_t[G : 2 * G, :1],
        ).wait_op(s_v, 3, "sem-ge").then_inc(s_s, 1)
        # z = sigmoid(gxz + gh_z): src 0:32, bias at 96:128
        scalar.wait_ge(s_s, 2)  # sem edge for n_t (same-engine RAW)
        scalar.activation(
            z_t[:],
            gxz[:, :1],
            mybir.ActivationFunctionType.Sigmoid,
            bias=gh[3 * G : 4 * G, :1],
        ).wait_op(s_v, 4, "sem-ge").then_inc(s_s, 1)

        vector.tensor_tensor(
            out=d_t[:], in0=hs_view, in1=n_t[:], op=mybir.AluOpType.subtract
        ).wait_op(s_s, 2, "sem-ge").then_inc(s_ve, 1)  # n ready (tanh 2nd inc)
        vector.wait_ge(s_ve, 1)  # sem edge for d_t (same-engine RAW)
        vector.scalar_tensor_tensor(
            out=out_sb[:],
            in0=d_t[:, :1].to_broadcast([G, 128]),
            scalar=z_t[:, :1],
            in1=n_t[:, :1].to_broadcast([G, 128]),
            op0=mybir.AluOpType.mult,
            op1=mybir.AluOpType.add,
        ).wait_op(s_s, 3, "sem-ge").then_inc(s_v, 1)

        # No completion wait: the DMA lands during the walrus teardown.
        sync.dma_start(out_d[:], out_sb[:]).wait_op(s_v, 5, "sem-ge").then_inc(
            s_out, 16
        )

    return nc


def shard_inputs(
    input, hidden, embedding, w_ih, w_hh, b_ih, b_hh
) -> list[dict[str, np.ndarray]]:
    """Host-side marshaling: slice/replicate full inputs into per-core maps."""
    idx = int(np.asarray(input).reshape(-1)[0])
    h = np.asarray(hidden, dtype=np.float32).reshape(HID)
    table = np.ascontiguousarray(np.asarray(embedding, dtype=np.float32))
    w_ih = np.asarray(w_ih, dtype=np.float32)
    w_hh = np.asarray(w_hh, dtype=np.float32)
    b_ih = np.asarray(b_ih, dtype=np.float32)
    b_hh = np.asarray(b_hh, dtype=np.float32)

    off_arr = np.array([[idx * HID]], dtype=np.int32)

    in_maps = []
    for k in range(NCORES):
        lo = G * k
        wd = np.zeros((128, WD), dtype=np.float32)
        # rows 0:32: h_n h-block | w_ih_z | hs
        wd[0:G, 0:HID] = w_hh[2 * HID + lo : 2 * HID + lo + G]
        wd[0:G, HID] = b_hh[2 * HID + lo : 2 * HID + lo + G]
        wd[0:G, HID + 2 : 2 * HID + 2] = h[None, :]
        wd[0:G, 2 * HID + 2 : 2 * HID + 4] = 1.0
        wd[0:G, 2 * HID + 4 : 3 * HID + 4] = w_ih[HID + lo : HID + lo + G]  # z
        wd[0:G, WD - 1] = h[lo : lo + G]  # hs
        # rows 32:64: w_ih_r | w_ih_n | b_ih_n
        wd[G : 2 * G, 0:HID] = w_ih[lo : lo + G]
        wd[G : 2 * G, HID : 2 * HID] = w_ih[2 * HID + lo : 2 * HID + lo + G]
        wd[G : 2 * G, 2 * HID] = b_ih[2 * HID + lo : 2 * HID + lo + G]
        # rows 64:96: r h-block
        wd[2 * G : 3 * G, 0:HID] = w_hh[lo : lo + G]
        wd[2 * G : 3 * G, HID] = b_ih[lo : lo + G]
        wd[2 * G : 3 * G, HID + 1] = b_hh[lo : lo + G]
        wd[2 * G : 3 * G, HID + 2 : 2 * HID + 2] = h[None, :]
        wd[2 * G : 3 * G, 2 * HID + 2 : 2 * HID + 4] = 1.0
        # rows 96:128: z h-block
        wd[3 * G : 4 * G, 0:HID] = w_hh[HID + lo : HID + lo + G]
        wd[3 * G : 4 * G, HID] = b_ih[HID + lo : HID + lo + G]
        wd[3 * G : 4 * G, HID + 1] = b_hh[HID + lo : HID + lo + G]
        wd[3 * G : 4 * G, HID + 2 : 2 * HID + 2] = h[None, :]
        wd[3 * G : 4 * G, 2 * HID + 2 : 2 * HID + 4] = 1.0

        in_maps.append({"table": table, "wd": wd, "off": off_arr})
    return in_maps


def unshard_output(results: list[dict[str, np.ndarray]]):
    h_new = np.concatenate(
        [np.asarray(results[k]["out"]).reshape(G, -1)[:, 0] for k in range(NCORES)]
    ).astype(np.float32)
    out = h_new.reshape(1, 1, HID)
    return out, out


def _get_program():
    global _cached
    if _cached is None:
        _cached = build_program()
    return _cached


def kernel(**inputs):
    from concourse.bass_utils import run_bass_kernel_spmd

    nc = _get_program()
    in_maps = shard_inputs(**inputs)
    res = run_bass_kernel_spmd(nc, in_maps, core_ids=list(range(NCORES)))
    return unshard_output(res.results)


def run_traced(**inputs):
    """Like kernel() but with NTFF tracing; returns (output, BassKernelResults)."""
    from concourse.bass_utils import run_bass_kernel_spmd

    nc = _get_program()
    in_maps = shard_inputs(**inputs)
    res = run_bass_kernel_spmd(nc, in_maps, core_ids=list(range(NCORES)), trace=True)
    return unshard_output(res.results), res


# revision 4
# speedup vs baseline: 1.0036x; 1.0036x over previous
"""GRU encoder step (embedding lookup + GRUCell, batch=1) on 8 TRN2 cores.

Sharding: core k computes hidden dims [32k, 32k+32); the host concatenates.

Key mechanisms (chosen from trace analysis):
- The looked-up embedding row is fetched with a register-dynamic HWDGE DMA:
  scalar reg_loads the element offset (idx*HID, host-marshaled i32) straight
  from DRAM and issues a ONE-DESCRIPTOR DMA of table[idx] into a single SBUF
  partition. One descriptor = one DMA engine = no 16-way completion
  rendezvous (a straggling engine was costing 1-2us on sprayed DMAs).
- The idle PE broadcasts the row across partitions: ones[1,64].T @ x[1,257]
  -> psum[64, 257] (col 256 is a 1.0 used to fold b_ih_n into the gxi
  contraction).
- ALL weights/biases/h ride in ONE [128, 773] DMA (first in flight, so its
  16-way completion is ahead of everything else):
    rows 0:32  : [w_hh_n | b_hh_n 0 | h 1 1 | w_ih_z | hs]
    rows 32:64 : [w_ih_r | w_ih_n | b_ih_n | 0...]
    rows 64:96 : [w_hh_r | b_ih_r b_hh_r | h 1 1 | 0...]
    rows 96:128: [w_hh_z | b_ih_z b_hh_z | h 1 1 | 0...]
  One 128-partition STT then yields gh[0:32]=ghn_full (tanh src),
  gh[32:64]=garbage (unused), gh[64:96]=r bias, gh[96:128]=z bias.
- x-side contractions read the PSUM broadcast: gxr/gxi on partitions 32:64,
  gxz on 0:32 (so z_t lands on 0:32 where the final blend runs).
- n = tanh(ghn*r + gxi) via ACT scale/bias APs (+32 partition-shifted).
- Consumer semaphore waits are fused onto the consuming instructions
  (standalone EVENT_SEMAPHORE waits cost ~80ns each).
- No nc.Block(): no entry branches, no bass exit barrier; the walrus
  epilogue's own barrier gates the teardown. Output DMA has no completion
  wait (lands during teardown).
"""

import os
import sys

import numpy as np

for _p in ("/opt/trn_rl_repo",):
    if _p not in sys.path and os.path.isdir(_p):
        sys.path.insert(0, _p)

import concourse.bass as bass
from concourse.ap import AP
from concourse import mybir

VOCAB = 100000
HID = 256
NCORES = 8
G = HID // NCORES  # 32
WD = 3 * HID + 5  # 773

_cached = None


def build_program():
    nc = bass.Bass(
        "TRN2",
        target_bir_lowering=False,
        debug=False,
        enable_asserts=False,
        num_devices=NCORES,
    )
    f32 = mybir.dt.float32
    i32 = mybir.dt.int32

    table_t = nc.dram_tensor("table", [VOCAB, HID], f32, kind="ExternalInput")
    off_t = nc.dram_tensor("off", [1, 1], i32, kind="ExternalInput")
    wd_d = nc.dram_tensor("wd", [128, WD], f32, kind="ExternalInput").ap()
    # [G, 128] so each partition's DMA write is 512B (avoids sub-512B RMW);
    # host reads column 0.
    out_d = nc.dram_tensor("out", [G, 128], f32, kind="ExternalOutput").ap()

    wd_sb = nc.alloc_sbuf_tensor("wd_sb", [128, WD], f32).ap()
    bxa1 = nc.alloc_sbuf_tensor("bxa1", [1, HID + 1], f32).ap()
    ones = nc.alloc_sbuf_tensor("ones", [1, 2 * G], f32).ap()
    px = nc.alloc_psum_tensor("px", [2 * G, HID + 1], f32).ap()
    s1 = nc.alloc_sbuf_tensor("s1", [2 * G, HID], f32).ap()
    s1b = nc.alloc_sbuf_tensor("s1b", [2 * G, HID + 1], f32).ap()
    s2 = nc.alloc_sbuf_tensor("s2", [128, HID + 2], f32).ap()
    s3 = nc.alloc_sbuf_tensor("s3", [G, HID], f32).ap()
    gh = nc.alloc_sbuf_tensor("gh", [128, 1], f32).ap()
    gxr = nc.alloc_sbuf_tensor("gxr", [2 * G, 1], f32).ap()
    gxi = nc.alloc_sbuf_tensor("gxi", [2 * G, 1], f32).ap()
    gxz = nc.alloc_sbuf_tensor("gxz", [G, 1], f32).ap()
    r_t = nc.alloc_sbuf_tensor("r_t", [2 * G, 1], f32).ap()
    z_t = nc.alloc_sbuf_tensor("z_t", [G, 1], f32).ap()
    n_t = nc.alloc_sbuf_tensor("n_t", [G, 1], f32).ap()
    d_t = nc.alloc_sbuf_tensor("d_t", [G, 1], f32).ap()
    out_sb = nc.alloc_sbuf_tensor("out_sb", [G, 128], f32).ap()
    warm = nc.alloc_sbuf_tensor("warm", [G, 1], f32).ap()

    hs_view = wd_sb[0:G, WD - 1 : WD]

    off_reg = nc.scalar.alloc_register("offr")

    with (
        nc.semaphore() as s_gx,
        nc.semaphore() as s_wd,
        nc.semaphore() as s_m,
        nc.semaphore() as s_px,
        nc.semaphore() as s_v,
        nc.semaphore() as s_ve,
        nc.semaphore() as s_s,
        nc.semaphore() as s_out,
    ):
        sync = nc.sync
        gpsimd = nc.gpsimd
        scalar = nc.scalar
        vector = nc.vector
        tensor = nc.tensor

        # --- sync: the single merged weight DMA, first in flight ---
        sync.dma_start(wd_sb[:], wd_d[:]).then_inc(s_wd, 16)

        # --- gpsimd: ones for the PE broadcast + the 1.0 in bxa1 col 256 ---
        gpsimd.memset(ones[:], 1.0)
        gpsimd.memset(bxa1[0:1, HID : HID + 1], 1.0).then_inc(s_m, 1)

        # --- scalar: 1-descriptor gather on the uncontended Act queue ---
        scalar.reg_load(off_reg, off_t[0:1, 0:1])
        v = scalar.snap(off_reg, donate=True, min_val=0, max_val=(VOCAB - 1) * HID)
        dyn = AP(tensor=table_t.ap().tensor, offset=v, ap=[[0, 1], [1, HID]])
        scalar.dma_start(bxa1[0:1, 0:HID], dyn, single_packet=True).then_inc(
            s_gx, 16
        )
        # Warm the ACT function table while DMAs are in flight.
        const0 = nc.const_aps.aps[(mybir.dt.float32, 0.0)]
        scalar.activation(
            warm[:], const0[0:G, :1], mybir.ActivationFunctionType.Sigmoid
        )

        # --- tensor: broadcast x across 64 partitions via ones-matmul ---
        tensor.wait_ge(s_m, 1)  # ones ready (gather wait fused on matmul)
        # wait >=16: a DMA's completion is 16 per-engine sem packets and only
        # the data-carrying engine's packet is write-after-write ordered with
        # the data -- waiting >=1 races the 15 empty packets (observed
        # intermittent corruption).
        tensor.matmul(
            px[0 : 2 * G, 0 : HID + 1],
            ones[0:1, 0 : 2 * G],
            bxa1[0:1, 0 : HID + 1],
        ).wait_op(s_gx, 16, "sem-ge").then_inc(s_px, 1)

        # --- vector: h-side contraction for all gates in one 128-row STT ---
        vector.scalar_tensor_tensor(
            out=s2[:],
            in0=wd_sb[:, 0 : HID + 2],
            scalar=1.0,
            in1=wd_sb[:, HID + 2 : 2 * HID + 4],
            op0=mybir.AluOpType.mult,
            op1=mybir.AluOpType.mult,
            accum_out=gh[:],
        ).wait_op(s_wd, 16, "sem-ge").then_inc(s_v, 1)
        # x-side contraction, r rows (partitions 32:64, reads PSUM broadcast)
        vector.scalar_tensor_tensor(
            out=s1[G : 2 * G, 0:HID],
            in0=wd_sb[G : 2 * G, 0:HID],
            scalar=1.0,
            in1=px[G : 2 * G, 0:HID],
            op0=mybir.AluOpType.mult,
            op1=mybir.AluOpType.mult,
            accum_out=gxr[G : 2 * G, :],
        ).wait_op(s_px, 1, "sem-ge").then_inc(s_v, 1)

        # r = sigmoid(gxr + gh_r): src/out partitions 32:64, bias at 64:96
        scalar.activation(
            r_t[G : 2 * G, :],
            gxr[G : 2 * G, :1],
            mybir.ActivationFunctionType.Sigmoid,
            bias=gh[2 * G : 3 * G, :1],
        ).wait_op(s_v, 2, "sem-ge").then_inc(s_s, 1)

        # x-side contraction, i_n rows + b_ih_n via psum ones column
        vector.scalar_tensor_tensor(
            out=s1b[G : 2 * G, :],
            in0=wd_sb[G : 2 * G, HID : 2 * HID + 1],
            scalar=1.0,
            in1=px[G : 2 * G, 0 : HID + 1],
            op0=mybir.AluOpType.mult,
            op1=mybir.AluOpType.mult,
            accum_out=gxi[G : 2 * G, :],
        ).then_inc(s_v, 1)
        # x-side contraction, z rows (partitions 0:32)
        vector.scalar_tensor_tensor(
            out=s3[:],
            in0=wd_sb[0:G, 2 * HID + 4 : 3 * HID + 4],
            scalar=1.0,
            in1=px[0:G, 0:HID],
            op0=mybir.AluOpType.mult,
            op1=mybir.AluOpType.mult,
            accum_out=gxz[:],
        ).then_inc(s_v, 1)

        # n = tanh(ghn*r + gxi): src gh[0:32], scale r_t(+32), bias gxi(+32)
        scalar.wait_ge(s_s, 1)  # sem edge for r_t (same-engine RAW)
        scalar.activation(
            n_t[:],
            gh[0:G, :1],
            mybir.ActivationFunctionType.Tanh,
            bias=gxi[G : 2 * G, :1],
            scale=r_t[G : 2 * G, :1],
        ).wait_op(s_v, 3, "sem-ge").then_inc(s_s, 1)
        # z = sigmoid(gxz + gh_z): src 0:32, bias at 96:128
        scalar.wait_ge(s_s, 2)  # sem edge for n_t (same-engine RAW)
        scalar.activation(
            z_t[:],
            gxz[:, :1],
            mybir.ActivationFunctionType.Sigmoid,
            bias=gh[3 * G : 4 * G, :1],
        ).wait_op(s_v, 4, "sem-ge").then_inc(s_s, 1)

        vector.tensor_tensor(
            out=d_t[:], in0=hs_view, in1=n_t[:], op=mybir.AluOpType.subtract
        ).wait_op(s_s, 2, "sem-ge").then_inc(s_ve, 1)  # n ready (tanh 2nd inc)
        vector.wait_ge(s_ve, 1)  # sem edge for d_t (same-engine RAW)
        vector.scalar_tensor_tensor(
            out=out_sb[:],
            in0=d_t[:, :1].to_broadcast([G, 128]),
            scalar=z_t[:, :1],
            in1=n_t[:, :1].to_broadcast([G, 128]),
            op0=mybir.AluOpType.mult,
            op1=mybir.AluOpType.add,
        ).wait_op(s_s, 3, "sem-ge").then_inc(s_v, 1)

        # No completion wait: the DMA lands during the walrus teardown.
        sync.dma_start(out_d[:], out_sb[:]).wait_op(s_v, 5, "sem-ge").then_inc(
            s_out, 16
        )

    return nc


def shard_inputs(
    input, hidden, embedding, w_ih, w_hh, b_ih, b_hh
) -> list[dict[str, np.ndarray]]:
    """Host-side marshaling: slice/replicate full inputs into per-core maps."""
    idx = int(np.asarray(input).reshape(-1)[0])
    h = np.asarray(hidden, dtype=np.float32).reshape(HID)
    table = np.ascontiguousarray(np.asarray(embedding, dtype=np.float32))
    w_ih = np.asarray(w_ih, dtype=np.float32)
    w_hh = np.asarray(w_hh, dtype=np.float32)
    b_ih = np.asarray(b_ih, dtype=np.float32)
    b_hh = np.asarray(b_hh, dtype=np.float32)

    off_arr = np.array([[idx * HID]], dtype=np.int32)

    in_maps = []
    for k in range(NCORES):
        lo = G * k
        wd = np.zeros((128, WD), dtype=np.float32)
        # rows 0:32: h_n h-block | w_ih_z | hs
        wd[0:G, 0:HID] = w_hh[2 * HID + lo : 2 * HID + lo + G]
        wd[0:G, HID] = b_hh[2 * HID + lo : 2 * HID + lo + G]
        wd[0:G, HID + 2 : 2 * HID + 2] = h[None, :]
        wd[0:G, 2 * HID + 2 : 2 * HID + 4] = 1.0
        wd[0:G, 2 * HID + 4 : 3 * HID + 4] = w_ih[HID + lo : HID + lo + G]  # z
        wd[0:G, WD - 1] = h[lo : lo + G]  # hs
        # rows 32:64: w_ih_r | w_ih_n | b_ih_n
        wd[G : 2 * G, 0:HID] = w_ih[lo : lo + G]
        wd[G : 2 * G, HID : 2 * HID] = w_ih[2 * HID + lo : 2 * HID + lo + G]
        wd[G : 2 * G, 2 * HID] = b_ih[2 * HID + lo : 2 * HID + lo + G]
        # rows 64:96: r h-block
        wd[2 * G : 3 * G, 0:HID] = w_hh[lo : lo + G]
        wd[2 * G : 3 * G, HID] = b_ih[lo : lo + G]
        wd[2 * G : 3 * G, HID + 1] = b_hh[lo : lo + G]
        wd[2 * G : 3 * G, HID + 2 : 2 * HID + 2] = h[None, :]
        wd[2 * G : 3 * G, 2 * HID + 2 : 2 * HID + 4] = 1.0
        # rows 96:128: z h-block
        wd[3 * G : 4 * G, 0:HID] = w_hh[HID + lo : HID + lo + G]
        wd[3 * G : 4 * G, HID] = b_ih[HID + lo : HID + lo + G]
        wd[3 * G : 4 * G, HID + 1] = b_hh[HID + lo : HID + lo + G]
        wd[3 * G : 4 * G, HID + 2 : 2 * HID + 2] = h[None, :]
        wd[3 * G : 4 * G, 2 * HID + 2 : 2 * HID + 4] = 1.0

        in_maps.append({"table": table, "wd": wd, "off": off_arr})
    return in_maps


def unshard_output(results: list[dict[str, np.ndarray]]):
    h_new = np.concatenate(
        [np.asarray(results[k]["out"]).reshape(G, -1)[:, 0] for k in range(NCORES)]
    ).astype(np.float32)
    out = h_new.reshape(1, 1, HID)
    return out, out


def _get_program():
    global _cached
    if _cached is None:
        _cached = build_program()
    return _cached


def kernel(**inputs):
    from concourse.bass_utils import run_bass_kernel_spmd

    nc = _get_program()
    in_maps = shard_inputs(**inputs)
    res = run_bass_kernel_spmd(nc, in_maps, core_ids=list(range(NCORES)))
    return unshard_output(res.results)


def run_traced(**inputs):
    """Like kernel() but with NTFF tracing; returns (output, BassKernelResults)."""
    from concourse.bass_utils import run_bass_kernel_spmd

    nc = _get_program()
    in_maps = shard_inputs(**inputs)
    res = run_bass_kernel_spmd(nc, in_maps, core_ids=list(range(NCORES)), trace=True)
    return unshard_output(res.results), res
